# revision 11
# baseline (speedup 1.0000x reference)
"""Trainium2 Bass kernel for nn_BaseSegmentTree (2-layer GNN over a fixed
segment-tree graph).  B=8 samples -> 8 NeuronCores, one sample per core.

Layout on device: feature-major [D=128 partitions, N=2048 nodes free].

v2 design (vs baseline):
  * Internal-node aggregation (subtree sums, dst 1..1023) moved OFF the PE
    onto a DVE tree recursion  T_v = A_{v+1} + pairsum(T_{v+1}),
    A_{v+1} = pairsum(g at level v+1), with per-level 1/deg scalars on
    GpSimd.  This removes ~6400 PE cycles + 32 matmuls per layer and
    shrinks the fp8 operand pack from 8704 to ~4224 columns.
  * Only leaf-dst window rows (1024..2047) remain as block-sparse fp8
    matmuls (24 chunks / 7936 rows per layer).
  * Weight matmuls done in two passes per layer (wroot right after each
    bank's gelu, wnei after agg) accumulating into the same PSUM banks.
  * LN: mean-centering via Cmat PE matmul; variance via 16 selector
    matmuls into [16,128] PSUM; rsqrt = int-hack + 1 Newton step on DVE;
    rstd broadcast via 16 selector matmuls (all as in baseline).
  * PE kept warm through the rsqrt barrier with a few dummy matmuls so
    the HAM clock gate never re-throttles mid-layer.
  * DMA diet: no invdeg slab (built on device from a [1,1024] row via
    two K=1 outer-product matmuls), no smap (per-level scalar ops).
"""

import sys

sys.path.insert(0, "/opt/trn_rl_repo")

import numpy as np
import ml_dtypes
from contextlib import ExitStack

import concourse.bass as bass
import concourse.bacc as bacc
import concourse.tile as tile
import concourse.mybir as mybir
from concourse.bass_utils import run_bass_kernel_spmd

FP32 = mybir.dt.float32
BF16 = mybir.dt.bfloat16
FP8 = mybir.dt.float8e4
I32 = mybir.dt.int32
AF = mybir.ActivationFunctionType
OP = mybir.AluOpType

DEPTH = 10
LEAF = 2**DEPTH          # 1024
NODE_NUM = 2 * LEAF - 1  # 2047
NN = NODE_NUM + 1        # 2048 nodes incl. global node 0
D = 128
B = 8

_CACHE = {}


# --------------------------------------------------------------------------
# host-side constant construction
# --------------------------------------------------------------------------

def _pos_enc():
    """enc [NN, D] float32, with the global-node -1.0 folded into column 0."""
    def sinusoid(pos, d):
        half = d // 2
        inv = np.exp(-np.arange(half, dtype=np.float64) * (np.log(10000.0) / half))
        ang = pos[:, None] * inv[None, :]
        return np.stack([np.sin(ang), np.cos(ang)], -1).reshape(pos.shape[0], d)

    idx = np.arange(NN, dtype=np.float64)
    vpos = np.floor(np.log2(np.where(idx == 0, 0.5, idx)))
    hpos = idx - np.exp2(vpos)
    enc = np.concatenate([sinusoid(hpos, D // 2), sinusoid(vpos, D // 2)], -1)
    enc = enc.astype(np.float32)
    enc[0] += -1.0
    return enc


def _build_counts(edge_index):
    """Count matrix [NN, NN] (dst, src) and degree vector for one sample."""
    src = np.asarray(edge_index[0], np.int64)
    dst = np.asarray(edge_index[1], np.int64)
    sample = (dst // NN) == 0
    s0, d0 = src[sample] % NN, dst[sample] % NN
    C = np.zeros((NN, NN), np.float32)
    np.add.at(C, (d0, s0), 1.0)
    deg = np.maximum(C.sum(1), 1.0)
    return C, deg


def _pack_leaf(counts):
    """Pack nonzero 128x128 blocks of counts^T restricted to leaf dst rows
    (1024..2047) into a contiguous fp8 operand, content-deduplicated.
    Chunks are emitted in j-availability order (src blocks 8..15, then
    4..7, then 0..3 -- matching the bank pipeline order [2,3,1,0]).
    Chunk = (j, pack_off, width, dst_off(0..1023), start, stop)."""
    CT = counts.T  # [src, dst]
    nzb = {}
    for j in range(16):
        bs = []
        for b in range(8, 16):
            if np.any(CT[128 * j:128 * (j + 1), 128 * b:128 * (b + 1)]):
                bs.append(b)
        if bs:
            nzb[j] = bs
    j_order = list(range(8, 16)) + list(range(4, 8)) + list(range(0, 4))
    raw = []
    for j in j_order:
        bs = nzb.get(j, [])
        runs = []
        for b in bs:
            if runs and runs[-1][-1] == b - 1:
                runs[-1].append(b)
            else:
                runs.append([b])
        for run in runs:
            seg = []
            for b in run:
                # split at the PSUM-bank boundary (dst cols 1024..1535 vs
                # 1536..2047 i.e. blocks 8..11 vs 12..15)
                if seg and ((b - 8) // 4 != (seg[0] - 8) // 4):
                    raw.append((j, seg[0], len(seg)))
                    seg = []
                seg.append(b)
            if seg:
                raw.append((j, seg[0], len(seg)))
    # split segments so each chunk's dst blocks are uniformly fresh/stale
    written = set()
    raw2 = []
    for (j, b0, nb) in raw:
        seg = []
        seg_fresh = None
        for b in range(b0, b0 + nb):
            fresh = b not in written
            if seg and fresh != seg_fresh:
                raw2.append((j, seg[0], len(seg)))
                seg = []
            seg.append(b)
            seg_fresh = fresh
        if seg:
            raw2.append((j, seg[0], len(seg)))
        written.update(range(b0, b0 + nb))
    bank_touch = {}
    for idx, (j, b0, nb) in enumerate(raw2):
        bank_touch.setdefault((b0 - 8) // 4, []).append(idx)
    chunks = []
    packed = []
    col_pos = {}
    for idx, (j, b0, nb) in enumerate(raw2):
        bank = (b0 - 8) // 4
        st = bank_touch[bank][0] == idx
        sp = bank_touch[bank][-1] == idx
        blk = CT[128 * j:128 * (j + 1), 128 * b0:128 * (b0 + nb)]
        w = 128 * nb
        ckeys = [blk[:, i].tobytes() for i in range(w)]
        o = None
        for pos in col_pos.get(ckeys[0], []):
            if pos + w <= len(packed) and all(
                    packed[pos + i] == ckeys[i] for i in range(1, w)):
                o = pos
                break
        if o is None:
            o = len(packed)
            for i, ck in enumerate(ckeys):
                col_pos.setdefault(ck, []).append(o + i)
                packed.append(ck)
        chunks.append((j, o, w, 128 * (b0 - 8), st, sp))
    WT = np.frombuffer(b"".join(packed), dtype=np.float32).reshape(
        len(packed), 128).T.astype(ml_dtypes.float8_e4m3)
    return np.ascontiguousarray(WT), chunks


# --------------------------------------------------------------------------
# device program
# --------------------------------------------------------------------------

# cstbf column layout (bf16): enc | ident | w_nei | w_root | ones8 | cmat
def _cb_layout(n_layers):
    o = {}
    o["enc"] = 0
    o["ident"] = NN
    o["wnei"] = NN + 128
    o["wroot"] = NN + 128 + 128 * n_layers
    o["ones8"] = NN + 128 + 256 * n_layers
    o["cmat"] = o["ones8"] + 256
    o["end"] = o["cmat"] + 128
    return o


def _build_program(pack_cols, chunks, n_layers, gamma_trivial, beta_trivial,
                   bnei_trivial):
    nc = bacc.Bacc("TRN2", target_bir_lowering=False, debug=False,
                   num_devices=B)

    CB = _cb_layout(n_layers)
    elem_d = nc.dram_tensor("elem", [128, LEAF], BF16, kind="ExternalInput").ap()
    cst32_d = nc.dram_tensor("cst32", [128, 3 * n_layers], FP32,
                             kind="ExternalInput").ap()
    cstbf_d = nc.dram_tensor("cstbf", [128, CB["end"]], BF16,
                             kind="ExternalInput").ap()
    wt_d = nc.dram_tensor("wtf8", [128, pack_cols], FP8,
                          kind="ExternalInput").ap()
    sel_d = nc.dram_tensor("selbf", [16, NN], BF16,
                           kind="ExternalInput").ap()
    invd_d = nc.dram_tensor("invd", [1, NN], BF16,
                            kind="ExternalInput").ap()
    out_d = nc.dram_tensor("out", [128, NN], FP32, kind="ExternalOutput").ap()

    MAGIC = 0x5F3759DF
    L = n_layers

    with tile.TileContext(nc) as tc, ExitStack() as ctx:
        cpool = ctx.enter_context(tc.tile_pool(name="const", bufs=1))
        wpool = ctx.enter_context(tc.tile_pool(name="work", bufs=1))
        spool = ctx.enter_context(tc.tile_pool(name="small", bufs=1))
        bpool = ctx.enter_context(tc.tile_pool(name="pbank", bufs=5, space="PSUM"))
        apool = ctx.enter_context(tc.tile_pool(name="pagg", bufs=2, space="PSUM"))
        vpool = ctx.enter_context(tc.tile_pool(name="pvar", bufs=1, space="PSUM"))

        # ---- input DMAs: critical pieces first ----
        e_sb = cpool.tile([128, LEAF], BF16, tag="e_sb")
        cstbf = cpool.tile([128, CB["end"]], BF16, tag="cstbf")
        cst32 = cpool.tile([128, 3 * L], FP32, tag="cst32")
        wt_sb = cpool.tile([128, pack_cols], FP8, tag="wt_sb")
        sel_sb = cpool.tile([16, NN], BF16, tag="sel_sb")
        invd_row = cpool.tile([1, NN], BF16, tag="invd_row")

        nc.sync.dma_start(out=e_sb[:], in_=elem_d[:])
        nc.scalar.dma_start(out=cstbf[:, LEAF:NN], in_=cstbf_d[:, LEAF:NN])
        nc.gpsimd.dma_start(out=cstbf[:, 0:LEAF], in_=cstbf_d[:, 0:LEAF])
        nc.sync.dma_start(out=cstbf[:, NN:], in_=cstbf_d[:, NN:])
        nc.scalar.dma_start(out=sel_sb[:], in_=sel_d[:])
        nc.scalar.dma_start(out=invd_row[:], in_=invd_d[:])
        nc.scalar.dma_start(out=cst32[:], in_=cst32_d[:])
        tw = (pack_cols // 2 + 127) & ~127
        nc.gpsimd.dma_start(out=wt_sb[:, 0:tw], in_=wt_d[:, 0:tw])
        nc.sync.dma_start(out=wt_sb[:, tw:], in_=wt_d[:, tw:])

        enc = cstbf[:, 0:NN]
        ident = cstbf[:, CB["ident"]:CB["ident"] + 128]
        wnei = lambda l: cstbf[:, CB["wnei"] + 128 * l:CB["wnei"] + 128 * (l + 1)]
        wroot = lambda l: cstbf[:, CB["wroot"] + 128 * l:CB["wroot"] + 128 * (l + 1)]
        ones8 = cstbf[:, CB["ones8"]:CB["ones8"] + 256]
        Cmat = cstbf[:, CB["cmat"]:CB["cmat"] + 128]
        WT = wt_sb
        bnei_col = lambda l: cst32[:, l:l + 1]
        gam_col = lambda l: cst32[:, L + l:L + l + 1]
        bet_col = lambda l: cst32[:, 2 * L + l:2 * L + l + 1]

        # ---- ACT table warm (square + gelu sets) during DMA window ----
        dummy = spool.tile([128, 8], BF16, tag="dummy")
        nc.vector.memset(dummy[:], 0.0)
        nc.scalar.activation(dummy[:], dummy[:], AF.Square)
        nc.scalar.activation(dummy[:], dummy[:], AF.Gelu)

        # ---- PE warm-up during the input DMA window ----
        wtile = spool.tile([128, 512], BF16, tag="wtile")
        onescol = spool.tile([1, 128], BF16, tag="onescol")
        nc.vector.memset(wtile[:], 0.0)
        nc.vector.memset(onescol[:], 1.0)
        warm_ps = apool.tile([128, 512], FP32, tag="agg", name="warm")
        for _ in range(10):
            nc.tensor.matmul(warm_ps[:], wtile[:, 0:128], wtile[:],
                             start=True, stop=True)

        # ---- scale slab [128, NN] (per-level 1/deg for internal nodes,
        # 1/deg for leaves) built from the [1, NN] row via four K=1
        # outer-product matmuls (stationary = ones column) ----
        scl_sb = wpool.tile([128, NN], BF16, tag="scl")
        ip = {}
        for half in range(4):
            ip[half] = apool.tile([128, 512], FP32, tag="agg", name=f"ip{half}")
            nc.tensor.matmul(ip[half][:], onescol[:],
                             invd_row[0:1, 512 * half:512 * (half + 1)],
                             start=True, stop=True)
            nc.scalar.copy(scl_sb[:, 512 * half:512 * (half + 1)],
                           ip[half][:])

        # ---- tree compression -> x = node_feat + enc ----
        x_sb = wpool.tile([128, NN], BF16, tag="x")
        S = wpool.tile([128, LEAF], FP32, tag="S")
        ev = e_sb.rearrange("p (n t) -> p n t", t=2)
        nc.vector.tensor_add(S[:, 512:1024], ev[:, :, 0], ev[:, :, 1])
        for v in range(8, -1, -1):
            lo, hi = 1 << v, 1 << (v + 1)
            sv = S[:, hi:2 * hi].rearrange("p (n t) -> p n t", t=2)
            nc.vector.tensor_add(S[:, lo:hi], sv[:, :, 0], sv[:, :, 1])
        nc.vector.tensor_add(x_sb[:, LEAF:NN], e_sb[:], enc[:, LEAF:NN])
        for v in range(9, -1, -1):
            lo, hi = 1 << v, 1 << (v + 1)
            nc.vector.scalar_tensor_tensor(
                out=x_sb[:, lo:hi], in0=S[:, lo:hi], scalar=float(2.0 ** (v - 10)),
                in1=enc[:, lo:hi], op0=OP.mult, op1=OP.add)
        nc.vector.tensor_copy(x_sb[:, 0:1], enc[:, 0:1])

        xout = wpool.tile([128, NN], FP32, tag="xout")
        d_sb = wpool.tile([128, NN], BF16, tag="d")
        sq_sb = wpool.tile([128, NN], BF16, tag="sq")
        h_sb = wpool.tile([128, NN], BF16, tag="h")
        g_sb = wpool.tile([128, NN], BF16, tag="g")
        gT = wpool.tile([128, NN], BF16, tag="gT")
        agg_sb = wpool.tile([128, NN], BF16, tag="agg_sb")
        # chain buffers (fp32): T_v at cumulative offsets, A_v likewise
        Tbuf = wpool.tile([128, 1024], FP32, tag="Tbuf")
        Abuf = wpool.tile([128, 512], FP32, tag="Abuf")
        a_off = {9: 0}
        for v in range(8, 0, -1):
            a_off[v] = a_off[v + 1] + (1 << (v + 1)) // 2

        def Tv(v):
            return Tbuf[:, (1 << v):(1 << (v + 1))]

        def Av(v):
            return Abuf[:, a_off[v]:a_off[v] + (1 << v) // 2]

        corder = [2, 3, 1, 0]
        jgroup = {3: list(range(8, 16)), 1: list(range(4, 8)),
                  0: list(range(0, 4)), 2: []}

        # ---- layers ----
        for l in range(L):
            # wave A: centering + squares + variance selectors
            d_ps = {}
            var_ps = vpool.tile([16, 128], FP32, tag="var", name=f"var{l}")
            for ci, c in enumerate(corder):
                sl = slice(512 * c, 512 * (c + 1))
                d_ps[c] = bpool.tile([128, 512], FP32, tag="bank",
                                     name=f"dps{l}_{c}")
                nc.tensor.matmul(d_ps[c][:], Cmat[:], x_sb[:, sl],
                                 start=True, stop=True)
                nc.scalar.activation(sq_sb[:, sl], d_ps[c][:], AF.Square)
                nc.scalar.copy(d_sb[:, sl], d_ps[c][:])
                for k in range(4):
                    cc = 4 * c + k
                    nc.tensor.matmul(
                        var_ps[:], ones8[:, 16 * cc:16 * (cc + 1)],
                        sq_sb[:, 128 * cc:128 * (cc + 1)],
                        start=(ci == 0 and k == 0), stop=(ci == 3 and k == 3),
                        skip_group_check=True)

            # keep the PE busy through the rsqrt barrier (HAM stays warm)
            dum = apool.tile([128, 512], FP32, tag="agg", name=f"dum{l}")
            dum2 = apool.tile([128, 512], FP32, tag="agg", name=f"dum2{l}")
            for _ in range(5):
                nc.tensor.matmul(dum[:], wtile[:, 0:128], wtile[:],
                                 start=True, stop=True)

            # rstd = rsqrt(var): bit-hack seed + one Newton step (batched)
            v_sb = spool.tile([16, 128], FP32, tag="v")
            y_sb = spool.tile([16, 128], FP32, tag="y")
            w_sb = spool.tile([16, 128], FP32, tag="w")
            p_sb = spool.tile([16, 128], FP32, tag="p")
            rstd_bf = spool.tile([16, 128], BF16, tag="rstd")
            nc.scalar.copy(v_sb[:], var_ps[:])
            nc.vector.tensor_scalar(out=w_sb.bitcast(I32)[:],
                                    in0=v_sb.bitcast(I32)[:],
                                    scalar1=1, scalar2=-1,
                                    op0=OP.logical_shift_right,
                                    op1=OP.bitwise_xor)
            nc.vector.tensor_scalar(out=y_sb.bitcast(I32)[:],
                                    in0=w_sb.bitcast(I32)[:],
                                    scalar1=MAGIC + 1, scalar2=None, op0=OP.add)
            nc.vector.tensor_mul(w_sb[:], v_sb[:], y_sb[:])
            nc.vector.tensor_mul(p_sb[:], w_sb[:], y_sb[:])
            nc.vector.tensor_scalar(out=p_sb[:], in0=p_sb[:], scalar1=-0.5,
                                    scalar2=1.5, op0=OP.mult, op1=OP.add)
            nc.vector.tensor_mul(rstd_bf[:], y_sb[:], p_sb[:])

            A0 = apool.tile([128, 512], FP32, tag="agg", name=f"A0_{l}")
            A1 = apool.tile([128, 512], FP32, tag="agg", name=f"A1_{l}")

            def bcast(c):
                r = bpool.tile([128, 512], FP32, tag="bank", name=f"rps{l}_{c}")
                for q in range(4):
                    rr = 4 * c + q
                    nc.tensor.matmul(r[:, 128 * q:128 * (q + 1)],
                                     sel_sb[:, 128 * rr:128 * (rr + 1)],
                                     rstd_bf[:], start=(q == 0), stop=(q == 3),
                                     skip_group_check=True)
                sl = slice(512 * c, 512 * (c + 1))
                nc.vector.tensor_mul(h_sb[:, sl], d_sb[:, sl], r[:])
                if not (gamma_trivial and beta_trivial):
                    nc.vector.tensor_scalar(out=h_sb[:, sl], in0=h_sb[:, sl],
                                            scalar1=gam_col(l),
                                            scalar2=bet_col(l),
                                            op0=OP.mult, op1=OP.add)
                nc.scalar.activation(g_sb[:, sl], h_sb[:, sl], AF.Gelu)

            def transp(c):
                for q in range(4):
                    j = 4 * c + q
                    t_ps = bpool.tile([128, 128], BF16, tag="bank",
                                      name=f"tp{l}_{j}")
                    nc.tensor.transpose(t_ps[:], g_sb[:, 128 * j:128 * (j + 1)],
                                        ident)
                    if q % 2 == 0:
                        nc.scalar.copy(gT[:, 128 * j:128 * (j + 1)], t_ps[:])
                    else:
                        nc.vector.tensor_copy(gT[:, 128 * j:128 * (j + 1)],
                                              t_ps[:])

            def agg_chunks(js):
                for (j, off, width, dstoff, st, sp) in chunks:
                    if j not in js:
                        continue
                    bank = dstoff // 512
                    boff = dstoff - 512 * bank
                    tgt = A0 if bank == 0 else A1
                    nc.tensor.matmul(tgt[:, boff:boff + width],
                                     gT[:, 128 * j:128 * (j + 1)],
                                     WT[:, off:off + width],
                                     start=st, stop=sp,
                                     skip_group_check=True)

            bcast(2)
            bcast(3)
            # chain feed: T9 = pairsum of leaf g (banks 2,3)
            gv = g_sb[:, 1024:2048].rearrange("p (n t) -> p n t", t=2)
            nc.vector.tensor_add(Tv(9), gv[:, :, 0], gv[:, :, 1])
            bcast(1)
            gv = g_sb[:, 512:1024].rearrange("p (n t) -> p n t", t=2)
            nc.gpsimd.tensor_add(Av(9), gv[:, :, 0], gv[:, :, 1])
            nc.vector.tensor_mul(agg_sb[:, 512:1024], Tv(9),
                                 scl_sb[:, 512:1024])
            for _ in range(3):
                nc.tensor.matmul(dum2[:], wtile[:, 0:128], wtile[:],
                                 start=True, stop=True)
            transp(2)
            transp(3)
            agg_chunks(range(8, 16))
            bcast(0)
            gv = g_sb[:, 256:512].rearrange("p (n t) -> p n t", t=2)
            nc.gpsimd.tensor_add(Av(8), gv[:, :, 0], gv[:, :, 1])
            transp(1)
            agg_chunks(range(4, 8))
            for v in range(7, 0, -1):
                lo, hi = 1 << v, 1 << (v + 1)
                gv = g_sb[:, lo:hi].rearrange("p (n t) -> p n t", t=2)
                nc.gpsimd.tensor_add(Av(v), gv[:, :, 0], gv[:, :, 1])
            nc.gpsimd.memset(agg_sb[:, 0:1], 0.0)
            transp(0)
            agg_chunks(range(0, 4))

            # internal chain: T_v = A_{v+1} + pairsum(T_{v+1}), v=8..0
            for v in range(8, -1, -1):
                tv1 = Tv(v + 1).rearrange("p (n t) -> p n t", t=2)
                nc.vector.tensor_add(Tv(v), tv1[:, :, 0], tv1[:, :, 1])
                nc.vector.tensor_add(Tv(v), Tv(v), Av(v + 1))
                if v == 8:
                    nc.vector.tensor_mul(agg_sb[:, 256:512], Tbuf[:, 256:512],
                                         scl_sb[:, 256:512])
                if v == 4:
                    nc.vector.tensor_mul(agg_sb[:, 16:256], Tbuf[:, 16:256],
                                         scl_sb[:, 16:256])
            nc.vector.tensor_mul(agg_sb[:, 1:16], Tbuf[:, 1:16],
                                 scl_sb[:, 1:16])

            # leaf agg: scale by 1/deg while copying PSUM -> SBUF
            nc.vector.tensor_mul(agg_sb[:, 1024:1536], A0[:],
                                 scl_sb[:, 1024:1536])
            nc.vector.tensor_mul(agg_sb[:, 1536:2048], A1[:],
                                 scl_sb[:, 1536:2048])

            # wave C: weight matmuls + residual per bank
            for c in (1, 2, 3, 0):
                sl = slice(512 * c, 512 * (c + 1))
                W_ps = bpool.tile([128, 512], FP32, tag="bank",
                                  name=f"wps{l}_{c}")
                nc.tensor.matmul(W_ps[:], wroot(l), g_sb[:, sl],
                                 start=True, stop=False, skip_group_check=True)
                nc.tensor.matmul(W_ps[:], wnei(l), agg_sb[:, sl],
                                 start=False, stop=True, skip_group_check=True)
                xo = x_sb if l < L - 1 else xout
                if bnei_trivial:
                    nc.vector.tensor_add(xo[:, sl], W_ps[:], x_sb[:, sl])
                else:
                    nc.vector.scalar_tensor_tensor(
                        out=xo[:, sl], in0=W_ps[:], scalar=bnei_col(l),
                        in1=x_sb[:, sl], op0=OP.add, op1=OP.add)
                if l == L - 1:
                    eng = {1: nc.sync, 2: nc.scalar, 3: nc.gpsimd,
                           0: nc.sync}[c]
                    eng.dma_start(out=out_d[:, sl], in_=xout[:, sl])

    nc.compile()
    return nc


# --------------------------------------------------------------------------
# public entry point
# --------------------------------------------------------------------------

def _get_compiled(inputs):
    key = "prog"
    if key in _CACHE:
        return _CACHE[key]

    ln_gamma = np.asarray(inputs["ln_gamma"], np.float32)
    ln_beta = np.asarray(inputs["ln_beta"], np.float32)
    w_nei = np.asarray(inputs["w_nei"], np.float32)
    b_nei = np.asarray(inputs["b_nei"], np.float32)
    w_root = np.asarray(inputs["w_root"], np.float32)
    edge_index = np.asarray(inputs["edge_index"])
    n_layers = ln_gamma.shape[0]

    counts, deg = _build_counts(edge_index)
    WTpack, chunks = _pack_leaf(counts)
    pack_cols = WTpack.shape[1]
    enc = _pos_enc()

    gamma_trivial = bool(np.all(ln_gamma == 1.0))
    beta_trivial = bool(np.all(ln_beta == 0.0))
    bnei_trivial = bool(np.all(b_nei == 0.0))

    cst32 = np.zeros((128, 3 * n_layers), np.float32)
    for l in range(n_layers):
        cst32[:, l] = b_nei[l]
        cst32[:, n_layers + l] = ln_gamma[l]
        cst32[:, 2 * n_layers + l] = ln_beta[l]

    CB = _cb_layout(n_layers)
    cstbf = np.zeros((128, CB["end"]), ml_dtypes.bfloat16)
    cstbf[:, 0:NN] = enc.T
    cstbf[:, CB["ident"]:CB["ident"] + 128] = np.eye(128, dtype=np.float32)
    for l in range(n_layers):
        cstbf[:, CB["wnei"] + 128 * l:CB["wnei"] + 128 * (l + 1)] = \
            w_nei[l].astype(ml_dtypes.bfloat16)
        cstbf[:, CB["wroot"] + 128 * l:CB["wroot"] + 128 * (l + 1)] = \
            w_root[l].astype(ml_dtypes.bfloat16)
    for c in range(16):  # ones8: block c has column c = 1/128
        cstbf[:, CB["ones8"] + 16 * c + c] = 1.0 / 128.0
    cstbf[:, CB["cmat"]:CB["cmat"] + 128] = (
        np.eye(128, dtype=np.float32) - 1.0 / 128.0)

    selbf = np.zeros((16, NN), ml_dtypes.bfloat16)
    for r in range(16):
        selbf[r, 128 * r:128 * (r + 1)] = 1.0

    invd = np.zeros((1, NN), np.float32)
    invd[0, 1:2048] = 1.0 / deg[1:2048]
    invd = invd.astype(ml_dtypes.bfloat16)

    nc = _build_program(pack_cols, chunks, n_layers, gamma_trivial,
                        beta_trivial, bnei_trivial)
    consts = {"cst32": cst32, "cstbf": cstbf, "wtf8": WTpack,
              "selbf": selbf, "invd": invd}
    _CACHE[key] = (nc, consts)
    return _CACHE[key]


def _in_maps(elements, consts):
    maps = []
    for i in range(B):
        m = {"elem": np.ascontiguousarray(elements[i].T).astype(
            ml_dtypes.bfloat16)}
        m.update(consts)
        maps.append(m)
    return maps


def kernel(**inputs):
    elements = np.asarray(inputs["elements"], np.float32)  # [B, LEAF, D]
    nc, consts = _get_compiled(inputs)
    res = run_bass_kernel_spmd(nc, _in_maps(elements, consts),
                               core_ids=list(range(B)))
    out = np.stack([res.results[i]["out"].T for i in range(B)])
    return out.astype(np.float32)
